# revision 32
# baseline (speedup 1.0000x reference)
"""Trainium2 Bass kernel for iterated VQ codebook clustering (nn_Net_34900904247300).

reference:
    for r in 3 iterations:
        sim = (x @ W.T) / ||W_v||        # [B,T,1000]
        idx = argmax_v sim               # [B,T]
        a = W[idx]                       # gather
        a = softmax(a*x, -1) * a         # fused gating
        x = x - a
        anchors.append(a)
    return stack(anchors, 1)             # [B,3,T,D]

Sharding: data-parallel over batch (B=16 over 8 cores, 2 each); codebook
replicated. Each core processes 4096 tokens in 32 tiles of 128 partitions.

Structure: all (iteration, tile) steps are linearized into one software
pipeline with 2-step lookahead so the PE stream never stalls on the
per-tile gating chain:
  SA(s+2): transpose x-tile, split into f32r hi/lo      (PE, ACT, DVE)
  SB(s+1): 3-term f32r matmul, argmax, gather dispatch  (PE, DVE, GP)
  SC(s):   gating, output DMA, residual update          (DVE, ACT, GP)
softmax skips the max-subtraction (|g| <= ~25 so exp cannot overflow in
f32; result is identical up to f32 rounding).
"""
import numpy as np

import concourse.bass as bass
import concourse.bacc as bacc
import concourse.mybir as mybir
import concourse.tile as tile
from concourse.bass_utils import run_bass_kernel_spmd
from concourse.masks import make_identity

P = 128          # partitions / tokens per tile
D = 512          # feature dim
V = 1000         # codebook size
DK = D // P      # 4 contraction chunks
VC = 8           # codebook row chunks (7 full + 104)
N_ITER = 3
N_CORES = 8
TOK = 4096       # tokens per core
NT = TOK // P    # 32 token tiles per core
NS = N_ITER * NT # 96 linearized pipeline steps
F32 = mybir.dt.float32
F32R = mybir.dt.float32r
AF = mybir.ActivationFunctionType
ALU = mybir.AluOpType

# v-halves aligned to PSUM banks (512 f32 = 1 bank)
V_SPLITS = [(0, 512), (512, V - 512)]

N_TERMS = 3      # f32r split terms: 3 = exact, 2/1 = cheaper but flips argmaxes


def _build():
    nc = bacc.Bacc("TRN2", target_bir_lowering=False, debug=False,
                   num_devices=N_CORES)
    x_d = nc.dram_tensor("x", [TOK, D], F32, kind="ExternalInput")
    w_d = nc.dram_tensor("w", [V, D], F32, kind="ExternalInput")
    out_d = nc.dram_tensor("out", [N_ITER, TOK, D], F32, kind="ExternalOutput")

    with tile.TileContext(nc) as tc:
        with (
            tc.tile_pool(name="const", bufs=1) as const,
            tc.tile_pool(name="wconst", bufs=1) as wconst,
            tc.tile_pool(name="xs", bufs=1) as xs_pool,
            tc.tile_pool(name="xq", bufs=5) as xq,
            tc.tile_pool(name="work", bufs=3) as work,
            tc.tile_pool(name="small", bufs=6) as small,
            tc.tile_pool(name="ps_t", bufs=2, space="PSUM") as ps_t,
            tc.tile_pool(name="ps_s", bufs=3, space="PSUM") as ps_s,
        ):
            ident = const.tile([P, P], F32)
            make_identity(nc, ident)

            # ---------- preprocessing: normalized transposed codebook ----------
            F8 = mybir.dt.float8e4
            wnT_hi = wconst.tile([P, DK, V], F32R, tag="wnT_hi")    # wn_hi * 2048
            wnT_lo = wconst.tile([P, DK, V], F32R, tag="wnT_lo")    # wn_lo * 2048
            wnT_hi8 = wconst.tile([P, 2, 2, V], F8, tag="wnT_hi8")  # wn_hi * 16, k-pairs
            with tc.tile_pool(name="wprep", bufs=1) as wprep:
                w_vp = wprep.tile([P, VC, D], F32, tag="wvp")
                nc.vector.memset(w_vp[:], 1.0)
                for c in range(VC):
                    vlen = V - 7 * P if c == 7 else P
                    nc.sync.dma_start(out=w_vp[:vlen, c, :],
                                      in_=w_d[c * P : c * P + vlen, :])
                # norms along d (free dim)
                norms2 = small.tile([P, VC], F32, tag="n2")
                sq = wprep.tile([P, D], F32, tag="sq")
                for c in range(VC):
                    nc.vector.tensor_mul(sq[:], w_vp[:, c, :], w_vp[:, c, :])
                    nc.vector.reduce_sum(norms2[:, c : c + 1], sq[:],
                                         axis=mybir.AxisListType.X)
                norms = small.tile([P, VC], F32, tag="nrm")
                nc.scalar.sqrt(norms[:], norms2[:])
                inv = small.tile([P, VC], F32, tag="inv")
                nc.vector.reciprocal(inv[:], norms[:])
                wn_vp = wprep.tile([P, VC, D], F32, tag="wnvp")
                for c in range(VC):
                    nc.vector.tensor_scalar_mul(wn_vp[:, c, :], w_vp[:, c, :],
                                                inv[:, c : c + 1])
                # transpose -> [d_part, dk, v]
                wnT_f32 = wprep.tile([P, DK, V], F32, tag="wnTf")
                for c in range(VC):
                    vlen = V - 7 * P if c == 7 else P
                    for k in range(DK):
                        pt = ps_t.tile([P, P], F32, tag="pxt")
                        nc.tensor.transpose(pt[:, :vlen],
                                            wn_vp[:vlen, c, k * P : (k + 1) * P],
                                            ident[:vlen, :vlen])
                        nc.scalar.copy(wnT_f32[:, k, c * P : c * P + vlen],
                                       pt[:, :vlen])
                # hi = f32r(wnT); lo = f32r(wnT - hi); then scale in place
                nc.scalar.copy(wnT_hi[:], wnT_f32[:])
                nc.vector.tensor_sub(wnT_lo[:], wnT_f32[:], wnT_hi[:])
                for pr in range(2):
                    for j in range(2):
                        nc.scalar.activation(wnT_hi8[:, pr, j, :],
                                             wnT_hi[:, pr * 2 + j, :],
                                             AF.Copy, scale=16.0)
                nc.vector.tensor_scalar_mul(wnT_hi[:], wnT_hi[:], 2048.0)
                nc.vector.tensor_scalar_mul(wnT_lo[:], wnT_lo[:], 2048.0)

            # ---------- persistent x tiles ----------
            xs = []
            for ti in range(NT):
                xst = xs_pool.tile([P, D], F32, tag=f"xs{ti}")
                nc.sync.dma_start(out=xst[:], in_=x_d[ti * P : (ti + 1) * P, :])
                xs.append(xst)

            # ---------- software-pipelined main loop ----------
            # step s = r*NT + ti; SA 2 ahead, SB 1 ahead, SC current.
            st = [dict() for _ in range(NS)]

            def SA(s):
                ti = s % NT
                pxt = ps_t.tile([P, D], F32, tag="pxt")
                for k in range(DK):
                    nc.tensor.transpose(pxt[:, k * P : (k + 1) * P],
                                        xs[ti][:, k * P : (k + 1) * P],
                                        ident[:])
                xT_hi = xq.tile([P, DK, P], F32R, tag="xT_hi")
                nc.scalar.copy(xT_hi[:], pxt[:])
                st[s]["xT_hi"] = xT_hi
                if True:
                    xT_lo = xq.tile([P, DK, P], F32R, tag="xT_lo")
                    nc.vector.tensor_sub(xT_lo[:], pxt[:], xT_hi[:])
                    xT_lo8 = xq.tile([P, 2, 2, P], mybir.dt.float8e4,
                                     tag="xT_lo8")
                    for pr in range(2):
                        nc.scalar.activation(
                            xT_lo8[:, pr, :, :],
                            xT_lo[:, pr * 2 : (pr + 1) * 2, :],
                            AF.Copy, scale=128.0)
                    st[s]["xT_lo8"] = xT_lo8

            def SB(s):
                xT_hi = st[s].pop("xT_hi")
                xT_lo8 = st[s].pop("xT_lo8")
                st[s].pop("xT_lo", None)
                psim = ps_s.tile([P, V], F32, tag="psim")
                for n0, n1 in V_SPLITS:
                    for t, rt in enumerate((wnT_hi, wnT_lo)):
                        for k in range(DK):
                            nc.tensor.matmul(
                                psim[:, n0 : n0 + n1],
                                lhsT=xT_hi[:, k, :],
                                rhs=rt[:, k, n0 : n0 + n1],
                                start=(t == 0 and k == 0),
                                stop=False,
                            )
                    for pr in range(2):
                        nc.tensor.matmul(
                            psim[:, n0 : n0 + n1],
                            lhsT=xT_lo8[:, pr, :, :],
                            rhs=wnT_hi8[:, pr, :, n0 : n0 + n1],
                            start=False,
                            stop=(pr == 1),
                            perf_mode=mybir.MatmulPerfMode.DoubleRow,
                        )
                # argmax over v, straight from PSUM
                m8 = small.tile([P, 8], F32, tag="m8")
                nc.vector.max(out=m8[:], in_=psim[:])
                idx8 = small.tile([P, 8], mybir.dt.uint32, tag="idx8")
                nc.vector.max_index(idx8[:], m8[:], psim[:])
                ag = work.tile([P, D], F32, tag="ag")
                nc.gpsimd.indirect_dma_start(
                    out=ag[:], out_offset=None, in_=w_d[:],
                    in_offset=bass.IndirectOffsetOnAxis(ap=idx8[:, :1], axis=0),
                )
                st[s]["ag"] = ag

            def SC(s):
                r, ti = divmod(s, NT)
                ag = st[s].pop("ag")
                g = work.tile([P, D], F32, tag="g")
                nc.vector.tensor_mul(g[:], ag[:], xs[ti][:])
                # no max-subtraction: |g| is small enough that exp stays finite
                e = work.tile([P, D], F32, tag="e")
                ssum = small.tile([P, 1], F32, tag="ssum")
                nc.scalar.activation(e[:], g[:], AF.Exp, accum_out=ssum[:])
                rinv = small.tile([P, 1], F32, tag="rinv")
                nc.vector.reciprocal(rinv[:], ssum[:])
                aout = work.tile([P, D], F32, tag="aout")
                nc.vector.scalar_tensor_tensor(
                    out=aout[:], in0=e[:], scalar=rinv[:], in1=ag[:],
                    op0=ALU.mult, op1=ALU.mult,
                )
                nc.sync.dma_start(out=out_d[r, ti * P : (ti + 1) * P, :],
                                  in_=aout[:])
                if r < N_ITER - 1:
                    nc.gpsimd.tensor_sub(xs[ti][:], xs[ti][:], aout[:])

            SA(0)
            SA(1)
            SA(2)
            SA(3)
            SB(0)
            for s in range(NS):
                if s + 4 < NS:
                    SA(s + 4)
                if s + 1 < NS:
                    SB(s + 1)
                SC(s)

    nc.compile()
    return nc


_NC = None


def _get_nc():
    global _NC
    if _NC is None:
        _NC = _build()
    return _NC


def kernel(x: np.ndarray, embed_weight: np.ndarray) -> np.ndarray:
    x = np.ascontiguousarray(np.asarray(x, dtype=np.float32))
    w = np.ascontiguousarray(np.asarray(embed_weight, dtype=np.float32))
    B, T, Dd = x.shape
    assert (B, T, Dd) == (16, 2048, 512) and w.shape == (V, D)
    nc = _get_nc()
    xs = x.reshape(N_CORES, TOK, D)
    in_maps = [{"x": xs[i], "w": w} for i in range(N_CORES)]
    res = run_bass_kernel_spmd(nc, in_maps, core_ids=list(range(N_CORES)))
    outs = np.stack([res.results[i]["out"] for i in range(N_CORES)])
    # [8, 3, 4096, 512] -> [8, 3, 2, 2048, 512] -> [16, 3, 2048, 512]
    out = outs.reshape(N_CORES, N_ITER, 2, T, D).transpose(0, 2, 1, 3, 4)
    return np.ascontiguousarray(out.reshape(B, N_ITER, T, D))
